# revision 13
# baseline (speedup 1.0000x reference)
"""Trainium2 Bass kernel for nn_AuxNetwork (dense MLP with per-window first layer).

Sharding: expert-parallel over n_win — core c owns windows [c*32, (c+1)*32).
Its rows (all 64 batches x its 32 windows) = 2048 rows; every layer after
stage 1 is row-independent, so no collectives: each core runs the full MLP on
its rows.

Per layer (all GEMMs in float32r — TF32-like e8m11, full PE rate at N=512):
  - stationary operand (lhsT) = transposed activations [K=128 feat, M=128 rows]
  - moving operand (rhs) = W.T slices [128, 512]; bias added via a rank-1
    K=1 matmul (ones x bias) opening each PSUM accumulation group
  - LN stats row-major via bn_stats/bn_aggr straight off the PSUM tiles
  - pre-LN values PE-transposed back to feature-major into the next layer's
    stationary tensor; LN normalize + LeakyReLU applied feature-major in
    place (row stats broadcast via gpsimd partition_broadcast)
"""

import numpy as np

import concourse.bass as bass
import concourse.mybir as mybir
import concourse.tile as tile
from concourse import bacc
from concourse.bass_utils import run_bass_kernel_spmd
from concourse.masks import make_identity

F32 = mybir.dt.float32
F32R = mybir.dt.float32r
AF = mybir.ActivationFunctionType
OP = mybir.AluOpType

# Problem constants
B = 64
N_WIN = 256
WIN = 512
H = 1024
H1 = 2048
H2 = 1024
OUT = 3
EPS = 1e-5
SLOPE = 0.01

N_CORES = 8
NW_C = N_WIN // N_CORES          # 32 windows per core
ROWS = B * NW_C                  # 2048 rows per core, r = n_local*64 + b
RT = 2                           # row tiles
R = ROWS // RT                   # 1024 rows per row tile
NT = 512                         # matmul free-dim tile
FO4 = 512                        # fc4 padded output width


def round_f32r(a: np.ndarray) -> np.ndarray:
    """Round-to-nearest-even to e8m11 (fp32 with low 12 mantissa bits zero)."""
    u = np.ascontiguousarray(a, dtype=np.float32).view(np.uint32)
    half = np.uint32(1 << 11)
    lsb = (u >> np.uint32(12)) & np.uint32(1)
    u = u + (half - np.uint32(1)) + lsb
    u &= np.uint32(0xFFFFF000)
    return u.view(np.float32)


def _pack_wt(w: np.ndarray) -> np.ndarray:
    """W [FO, FI] -> wt [FI//128, 128, FO] with wt[fi, p, fo] = W[fo, fi*128+p]."""
    fo, fi = w.shape
    return np.ascontiguousarray(w.T.reshape(fi // 128, 128, fo))


# ----------------------------------------------------------------------------
# Program construction
# ----------------------------------------------------------------------------

class _Ctx:
    """Bundles nc, dram handles, pools, and shared tiles."""


def _declare_io(nc):
    d = {}
    d["xh"] = nc.dram_tensor("xh", [NW_C, 128, 4, B], F32R, kind="ExternalInput")
    d["w1t"] = nc.dram_tensor("w1t", [NW_C, 4, 128, H], F32R, kind="ExternalInput")
    d["b1t"] = nc.dram_tensor("b1t", [NW_C, H], F32R, kind="ExternalInput")
    d["w2t"] = nc.dram_tensor("w2t", [H // 128, 128, H1], F32R, kind="ExternalInput")
    d["b2t"] = nc.dram_tensor("b2t", [1, H1], F32R, kind="ExternalInput")
    d["wb0t"] = nc.dram_tensor("wb0t", [H1 // 128, 128, H1], F32R, kind="ExternalInput")
    d["bb0t"] = nc.dram_tensor("bb0t", [1, H1], F32R, kind="ExternalInput")
    d["wb1t"] = nc.dram_tensor("wb1t", [H1 // 128, 128, H1], F32R, kind="ExternalInput")
    d["bb1t"] = nc.dram_tensor("bb1t", [1, H1], F32R, kind="ExternalInput")
    d["w3t"] = nc.dram_tensor("w3t", [H1 // 128, 128, H2], F32R, kind="ExternalInput")
    d["b3t"] = nc.dram_tensor("b3t", [1, H2], F32R, kind="ExternalInput")
    d["w4t"] = nc.dram_tensor("w4t", [H2 // 128, 128, FO4], F32R, kind="ExternalInput")
    d["b4t"] = nc.dram_tensor("b4t", [1, FO4], F32R, kind="ExternalInput")
    d["ones"] = nc.dram_tensor("ones", [1, 128], F32R, kind="ExternalInput")
    # outputs, feature-major [F, ROWS] except o4 (row-major, tiny)
    d["o1"] = nc.dram_tensor("o1", [H, ROWS], F32R, kind="ExternalOutput")
    d["o2"] = nc.dram_tensor("o2", [H1, ROWS], F32R, kind="ExternalOutput")
    d["o3"] = nc.dram_tensor("o3", [H2, ROWS], F32R, kind="ExternalOutput")
    d["o4"] = nc.dram_tensor("o4", [ROWS, OUT], F32, kind="ExternalOutput")
    return d


def _compute_rn(cx, mv, m_extent, name):
    """mv [m, 2] = (mean, var) -> rn [m, 2] = (rstd, mean)."""
    nc = cx.nc
    rn = cx.small.tile([128, 2], F32, tag="rn", name=name, bufs=18)
    nc.scalar.activation(rn[:m_extent, 0:1], mv[:m_extent, 1:2], AF.Sqrt,
                         bias=cx.eps_sb[:m_extent, :], scale=1.0)
    nc.vector.reciprocal(rn[:m_extent, 0:1], rn[:m_extent, 0:1])
    nc.vector.tensor_copy(rn[:m_extent, 1:2], mv[:m_extent, 0:1])
    return rn


def _ln_apply(cx, rn_list, m_extent, srow, A, fo_chunks, out_d, rt, gam, bet,
              pref):
    """Assemble row-stat vectors, broadcast per half, normalize + affine +
    LeakyReLU in place on A, then optional DMA to out_d."""
    nc = cx.nc
    srow_s, srow_m = srow
    for i, rn in enumerate(rn_list):
        tp = cx.psum_tp.tile([1, 128], F32, tag="tp", name=f"tps_{pref}_{i}")
        nc.tensor.transpose(tp[:, :m_extent], rn[:m_extent, 0:1],
                            cx.identity[:m_extent, :m_extent])
        nc.vector.tensor_copy(
            srow_s[:, i * m_extent:(i + 1) * m_extent], tp[:, :m_extent])
        tpm = cx.psum_tp.tile([1, 128], F32, tag="tp", name=f"tpm_{pref}_{i}")
        nc.tensor.transpose(tpm[:, :m_extent], rn[:m_extent, 1:2],
                            cx.identity[:m_extent, :m_extent])
        nc.vector.tensor_copy(
            srow_m[:, i * m_extent:(i + 1) * m_extent], tpm[:, :m_extent])
    for h in range(R // NT):
        sbc = cx.bc.tile([128, NT], F32R, tag="sbc", name=f"sbc_{pref}_{h}",
                         bufs=1)
        mbc = cx.bc.tile([128, NT], F32R, tag="mbc", name=f"mbc_{pref}_{h}",
                         bufs=1)
        nc.gpsimd.partition_broadcast(sbc[:, :], srow_s[0:1, h * NT:(h + 1) * NT])
        nc.gpsimd.partition_broadcast(mbc[:, :], srow_m[0:1, h * NT:(h + 1) * NT])
        for fo in range(fo_chunks):
            sl = A[:, fo, h * NT:(h + 1) * NT]
            nc.vector.tensor_tensor(out=sl, in0=sl, in1=mbc[:, :], op=OP.subtract)
            nc.vector.tensor_tensor(out=sl, in0=sl, in1=sbc[:, :], op=OP.mult)
            if gam is not None:
                nc.vector.tensor_scalar(
                    out=sl, in0=sl, scalar1=gam[:, fo:fo + 1],
                    scalar2=bet[:, fo:fo + 1], op0=OP.mult, op1=OP.add)
            nc.scalar.activation(sl, sl, AF.Lrelu, bias=0.0, scale=1.0,
                                 alpha=SLOPE)
    if out_d is not None:
        for fo in range(fo_chunks):
            nc.sync.dma_start(
                out=out_d[fo * 128:(fo + 1) * 128, rt * R:(rt + 1) * R],
                in_=A[:, fo, :])


def _stage1(cx, rt, A1, srow, gam, bet):
    """Per-window Linear(512->1024) + LN + LeakyReLU -> A1 [128, 8, R], o1."""
    nc, d = cx.nc, cx.d
    n_no = H // NT  # 2
    rn_list = []
    for nl in range(R // B):  # 16 windows in this row tile
        n = rt * (R // B) + nl
        xt = cx.x.tile([128, 4, B], F32R, tag="xh", name=f"xh_{rt}_{nl}", bufs=1)
        nc.sync.dma_start(out=xt, in_=d["xh"][n, :, :, :])
        w1s = []
        for wi in range(4):
            w1 = cx.w1.tile([128, H], F32R, tag="w1", name=f"w1_{rt}_{nl}_{wi}",
                            bufs=4)
            nc.sync.dma_start(out=w1, in_=d["w1t"][n, wi, :, :])
            w1s.append(w1)

        stats = cx.small.tile([128, n_no, 6], F32, tag="stats",
                              name=f"st1_{rt}_{nl}", bufs=10)
        rmts = []
        for no in range(n_no):
            b1 = cx.bias.tile([1, NT], F32R, tag="bias", name=f"b1_{rt}_{nl}_{no}",
                              bufs=2)
            nc.sync.dma_start(out=b1, in_=d["b1t"][n:n + 1, no * NT:(no + 1) * NT])
            ps = cx.psum_g.tile([B, NT], F32, tag="gemm",
                                name=f"ps1_{rt}_{nl}_{no}", bufs=4)
            nc.tensor.matmul(ps[:, :], cx.ones_sb[:1, :B], b1[:, :],
                             start=True, stop=False)
            for wi in range(4):
                nc.tensor.matmul(ps[:, :], xt[:, wi, :],
                                 w1s[wi][:, no * NT:(no + 1) * NT],
                                 start=False, stop=(wi == 3))
            nc.vector.bn_stats(stats[:B, no, :], ps[:, :])
            rmt = cx.rm.tile([B, NT], F32, tag="rm1", name=f"rm1_{rt}_{nl}_{no}",
                             bufs=2)
            nc.scalar.activation(rmt[:, :], ps[:, :], AF.Identity, bias=0.0,
                                 scale=1.0)
            rmts.append(rmt)
        mv = cx.small.tile([128, 2], F32, tag="mv", name=f"mv1_{rt}_{nl}", bufs=6)
        nc.vector.bn_aggr(mv[:B, :], stats[:B, :, :])
        rn_list.append(_compute_rn(cx, mv, B, f"rn1_{rt}_{nl}"))

        tp = cx.psum_tp.tile([128, 8 * B], F32, tag="tp", name=f"tp1_{rt}_{nl}",
                             bufs=3)
        for j in range(8):
            nc.tensor.transpose(tp[:, j * B:(j + 1) * B],
                                rmts[j // 4][:, (j % 4) * 128:(j % 4 + 1) * 128],
                                cx.identity[:B, :B])
        nc.scalar.activation(
            A1[:, :, nl * B:(nl + 1) * B],
            tp[:, :].rearrange("p (f b) -> p f b", b=B),
            AF.Copy, bias=0.0, scale=1.0)

    _ln_apply(cx, rn_list, B, srow, A1, H // 128, d["o1"], rt, gam, bet,
              f"s1_{rt}")


def _dense_layer(cx, rt, A_in, fi_chunks, fo, w_name, b_name, A_out,
                 srow, out_d, ln, gam, bet, tag):
    """Generic row-major GEMM layer over row tile rt. If ln: LN+LeakyReLU into
    A_out [128, fo//128, R] (+ optional DMA of the normalized result). Else
    (fc4): bias only, DMA first OUT columns row-major."""
    nc, d = cx.nc, cx.d
    n_no = fo // NT
    n_rc = R // 128
    stats = []
    if ln:
        for rc in range(n_rc):
            stats.append(cx.small.tile([128, n_no, 6], F32, tag="stats",
                                       name=f"st_{tag}_{rt}_{rc}", bufs=10))

    for no in range(n_no):
        bt = cx.bias.tile([1, NT], F32R, tag="bias", name=f"bt_{tag}_{rt}_{no}",
                          bufs=2)
        nc.sync.dma_start(out=bt, in_=d[b_name][0:1, no * NT:(no + 1) * NT])
        wts = []
        for fi in range(fi_chunks):
            wt = cx.w.tile([128, NT], F32R, tag="wt",
                           name=f"wt_{tag}_{rt}_{no}_{fi}", bufs=16)
            nc.sync.dma_start(out=wt, in_=d[w_name][fi, :, no * NT:(no + 1) * NT])
            wts.append(wt)
        for rc in range(n_rc):
            ps = cx.psum_g.tile([128, NT], F32, tag="gemm",
                                name=f"ps_{tag}_{rt}_{no}_{rc}", bufs=4)
            nc.tensor.matmul(ps[:, :], cx.ones_sb[:1, :], bt[:, :],
                             start=True, stop=False)
            for fi in range(fi_chunks):
                nc.tensor.matmul(ps[:, :], A_in[:, fi, rc * 128:(rc + 1) * 128],
                                 wts[fi], start=False, stop=(fi == fi_chunks - 1))
            rmt = cx.rm.tile([128, NT], F32, tag="rm",
                             name=f"rm_{tag}_{rt}_{no}_{rc}", bufs=2)
            if ln:
                nc.vector.bn_stats(stats[rc][:, no, :], ps[:, :])
                nc.scalar.activation(rmt, ps[:, :], AF.Identity, bias=0.0, scale=1.0)
                tp = cx.psum_tp.tile([128, NT], F32, tag="tp",
                                     name=f"tp_{tag}_{rt}_{no}_{rc}", bufs=3)
                for j in range(4):
                    nc.tensor.transpose(tp[:, j * 128:(j + 1) * 128],
                                        rmt[:, j * 128:(j + 1) * 128],
                                        cx.identity[:, :])
                nc.scalar.activation(
                    A_out[:, no * 4:no * 4 + 4, rc * 128:(rc + 1) * 128],
                    tp[:, :].rearrange("p (f r) -> p f r", r=128),
                    AF.Copy, bias=0.0, scale=1.0)
            else:
                nc.scalar.activation(rmt, ps[:, :], AF.Identity, bias=0.0, scale=1.0)
                nc.sync.dma_start(
                    out=d["o4"][rt * R + rc * 128:rt * R + (rc + 1) * 128, :],
                    in_=rmt[:, :OUT])

    if ln:
        rn_list = []
        for rc in range(n_rc):
            mv = cx.small.tile([128, 2], F32, tag="mv", name=f"mv_{tag}_{rt}_{rc}",
                               bufs=6)
            nc.vector.bn_aggr(mv[:, :], stats[rc][:, :, :])
            rn_list.append(_compute_rn(cx, mv, 128, f"rn_{tag}_{rt}_{rc}"))
        _ln_apply(cx, rn_list, 128, srow, A_out, fo // 128, out_d, rt, gam,
                  bet, f"{tag}_{rt}")


def build_program(general_affine=False, repeat=1):
    nc = bacc.Bacc("TRN2", target_bir_lowering=False, debug=False)
    cx = _Ctx()
    cx.nc = nc
    d = _declare_io(nc)
    cx.d = d
    if general_affine:
        for nm, fo in (("1", H), ("2", H1), ("b0", H1), ("b1", H1), ("3", H2)):
            d[f"g{nm}"] = nc.dram_tensor(f"g{nm}", [128, fo // 128], F32,
                                         kind="ExternalInput")
            d[f"be{nm}"] = nc.dram_tensor(f"be{nm}", [128, fo // 128], F32,
                                          kind="ExternalInput")

    import contextlib
    with tile.TileContext(nc) as tc, contextlib.ExitStack() as ctx:
        cx.acts = ctx.enter_context(tc.tile_pool(name="acts", bufs=2))
        cx.w = ctx.enter_context(tc.tile_pool(name="wp", bufs=16))
        cx.w1 = ctx.enter_context(tc.tile_pool(name="w1p", bufs=4))
        cx.x = ctx.enter_context(tc.tile_pool(name="xp", bufs=1))
        cx.rm = ctx.enter_context(tc.tile_pool(name="rmp", bufs=3))
        cx.bias = ctx.enter_context(tc.tile_pool(name="biasp", bufs=2))
        cx.small = ctx.enter_context(tc.tile_pool(name="smallp", bufs=10))
        cx.bc = ctx.enter_context(tc.tile_pool(name="bcp", bufs=1))
        cx.singles = ctx.enter_context(tc.tile_pool(name="singles", bufs=1))
        cx.psum_g = ctx.enter_context(tc.tile_pool(name="psg", bufs=4, space="PSUM"))
        cx.psum_tp = ctx.enter_context(tc.tile_pool(name="pst", bufs=3, space="PSUM"))

        cx.identity = cx.singles.tile([128, 128], F32, name="identity")
        make_identity(nc, cx.identity)
        cx.ones_sb = cx.singles.tile([1, 128], F32R, name="ones_sb")
        nc.sync.dma_start(out=cx.ones_sb, in_=d["ones"][:, :])
        cx.eps_sb = cx.singles.tile([128, 1], F32, name="eps_sb")
        nc.vector.memset(cx.eps_sb, EPS)

        import contextlib as _cl
        loop_cm = tc.For_i(0, repeat, 1) if repeat > 1 else _cl.nullcontext()
        gams = {}
        if general_affine:
            for nm, fo in (("1", H), ("2", H1), ("b0", H1), ("b1", H1), ("3", H2)):
                g = cx.singles.tile([128, fo // 128], F32, name=f"gt{nm}")
                be = cx.singles.tile([128, fo // 128], F32, name=f"bet{nm}")
                nc.sync.dma_start(out=g, in_=d[f"g{nm}"][:, :])
                nc.sync.dma_start(out=be, in_=d[f"be{nm}"][:, :])
                gams[nm] = (g, be)
        else:
            gams = {nm: (None, None) for nm in ("1", "2", "b0", "b1", "3")}

        with loop_cm:
          for rt in range(RT):
            srow = (cx.bc.tile([1, R], F32R, tag="srow_s", name=f"srow_s_{rt}",
                               bufs=1),
                    cx.bc.tile([1, R], F32R, tag="srow_m", name=f"srow_m_{rt}",
                               bufs=1))

            A1 = cx.acts.tile([128, H // 128, R], F32R, tag="acts",
                              name=f"A1_{rt}")
            _stage1(cx, rt, A1, srow, *gams["1"])
            A2 = cx.acts.tile([128, H1 // 128, R], F32R, tag="acts",
                              name=f"A2_{rt}")
            _dense_layer(cx, rt, A1, H // 128, H1, "w2t", "b2t", A2,
                         srow, None, True, *gams["2"], tag="fc2")
            A3 = cx.acts.tile([128, H1 // 128, R], F32R, tag="acts",
                              name=f"A3_{rt}")
            _dense_layer(cx, rt, A2, H1 // 128, H1, "wb0t", "bb0t", A3,
                         srow, None, True, *gams["b0"], tag="b0")
            A4 = cx.acts.tile([128, H1 // 128, R], F32R, tag="acts",
                              name=f"A4_{rt}")
            _dense_layer(cx, rt, A3, H1 // 128, H1, "wb1t", "bb1t", A4,
                         srow, d["o2"], True, *gams["b1"], tag="b1")
            A5 = cx.acts.tile([128, H2 // 128, R], F32R, tag="acts",
                              name=f"A5_{rt}")
            _dense_layer(cx, rt, A4, H1 // 128, H2, "w3t", "b3t", A5,
                         srow, d["o3"], True, *gams["3"], tag="fc3")
            _dense_layer(cx, rt, A5, H2 // 128, FO4, "w4t", "b4t", None,
                         srow, d["o4"], False, None, None, tag="fc4")

    nc.compile()
    return nc


# ----------------------------------------------------------------------------
# Host side
# ----------------------------------------------------------------------------

_CACHE = {}


def prep_inputs(x, w1, b1, fc2_w, fc2_b, blk_w, blk_b, fc3_w, fc3_b,
                fc4_w, fc4_b):
    shared = {
        "w2t": round_f32r(_pack_wt(np.asarray(fc2_w))),
        "b2t": round_f32r(np.asarray(fc2_b)).reshape(1, -1),
        "wb0t": round_f32r(_pack_wt(np.asarray(blk_w)[0])),
        "bb0t": round_f32r(np.asarray(blk_b)[0]).reshape(1, -1),
        "wb1t": round_f32r(_pack_wt(np.asarray(blk_w)[1])),
        "bb1t": round_f32r(np.asarray(blk_b)[1]).reshape(1, -1),
        "w3t": round_f32r(_pack_wt(np.asarray(fc3_w))),
        "b3t": round_f32r(np.asarray(fc3_b)).reshape(1, -1),
        "ones": np.ones((1, 128), np.float32),
    }
    w4 = np.zeros((FO4, H2), np.float32)
    w4[:OUT, :] = np.asarray(fc4_w)
    b4 = np.zeros((FO4,), np.float32)
    b4[:OUT] = np.asarray(fc4_b)
    shared["w4t"] = round_f32r(_pack_wt(w4))
    shared["b4t"] = round_f32r(b4).reshape(1, -1)

    x = np.asarray(x)
    w1 = np.asarray(w1)
    b1 = np.asarray(b1)
    in_maps = []
    for c in range(N_CORES):
        m = dict(shared)
        xc = x[:, c * NW_C * WIN:(c + 1) * NW_C * WIN]
        xc = xc.reshape(B, NW_C, 4, 128)                 # [b, n, wi, p]
        m["xh"] = round_f32r(xc.transpose(1, 3, 2, 0))   # [n, p, wi, b]
        w1c = w1[c * NW_C:(c + 1) * NW_C].reshape(NW_C, H, 4, 128)
        m["w1t"] = round_f32r(w1c.transpose(0, 2, 3, 1))  # [n, wi, p, h]
        m["b1t"] = round_f32r(b1[c * NW_C:(c + 1) * NW_C])
        in_maps.append(m)
    return in_maps


def assemble_outputs(results):
    def fm_gather(key, F):
        full = np.empty((B, N_WIN, F), np.float32)
        for c, r in enumerate(results):
            a = r[key].reshape(F, NW_C, B)               # [f, n, b]
            full[:, c * NW_C:(c + 1) * NW_C, :] = a.transpose(2, 1, 0)
        return full

    o1 = fm_gather("o1", H)
    o2 = fm_gather("o2", H1)
    o3 = fm_gather("o3", H2)
    o4 = np.empty((B, N_WIN, OUT), np.float32)
    for c, r in enumerate(results):
        a = r["o4"].reshape(NW_C, B, OUT)
        o4[:, c * NW_C:(c + 1) * NW_C, :] = a.transpose(1, 0, 2)
    return o1, o2, o3, o4


def _ln_params_identity(ln1_g, ln1_b, ln2_g, ln2_b, blk_ln_g, blk_ln_b,
                        ln3_g, ln3_b):
    return (np.all(np.asarray(ln1_g) == 1) and np.all(np.asarray(ln1_b) == 0)
            and np.all(np.asarray(ln2_g) == 1) and np.all(np.asarray(ln2_b) == 0)
            and np.all(np.asarray(blk_ln_g) == 1)
            and np.all(np.asarray(blk_ln_b) == 0)
            and np.all(np.asarray(ln3_g) == 1) and np.all(np.asarray(ln3_b) == 0))


def kernel(x, w1, b1, ln1_g, ln1_b, fc2_w, fc2_b, ln2_g, ln2_b,
           blk_w, blk_b, blk_ln_g, blk_ln_b, fc3_w, fc3_b, ln3_g, ln3_b,
           fc4_w, fc4_b):
    general = not _ln_params_identity(ln1_g, ln1_b, ln2_g, ln2_b,
                                      blk_ln_g, blk_ln_b, ln3_g, ln3_b)
    key = ("prog", general)
    if key not in _CACHE:
        _CACHE[key] = build_program(general_affine=general)
    nc = _CACHE[key]

    in_maps = prep_inputs(x, w1, b1, fc2_w, fc2_b, blk_w, blk_b,
                          fc3_w, fc3_b, fc4_w, fc4_b)
    if general:
        def chunked(a, F):
            # [F] -> [128, F//128] with [p, fo] = a[fo*128+p]
            return np.ascontiguousarray(
                np.asarray(a, np.float32).reshape(F // 128, 128).T)
        extra = {}
        for nm, arr_g, arr_b, F in (
                ("1", ln1_g, ln1_b, H), ("2", ln2_g, ln2_b, H1),
                ("b0", np.asarray(blk_ln_g)[0], np.asarray(blk_ln_b)[0], H1),
                ("b1", np.asarray(blk_ln_g)[1], np.asarray(blk_ln_b)[1], H1),
                ("3", ln3_g, ln3_b, H2)):
            extra[f"g{nm}"] = chunked(arr_g, F)
            extra[f"be{nm}"] = chunked(arr_b, F)
        for m in in_maps:
            m.update(extra)

    res = run_bass_kernel_spmd(nc, in_maps, list(range(N_CORES)))
    return assemble_outputs(res.results)
